# revision 3
# baseline (speedup 1.0000x reference)
"""Trainium2 Bass kernel for the 8-level 1-D hash-grid embedding lookup.

Strategy
--------
All 8 levels have power-of-two resolutions (16..2048) whose grid knots all
lie on multiples of 1/2048, and no level ever indexes past row 2048 of its
2^19-row table (so the hash mask is a no-op).  The concatenated 16-feature
output is therefore a single piecewise-linear function of x with 2048
segments.  On the host we fuse the 8 tables into one [2048, 32] f32 table
G[k] = [f(k/2048) | f((k+1)/2048) - f(k/2048)]  (16 values + 16 deltas), and
the device kernel per point does:
    y = x * 2048 ; m = floor(y) ; t = y - m
    out = G[m, :16] + t * G[m, 16:]
i.e. ONE 128-byte indirect-DMA gather per point instead of 16 tiny ones.

Data-parallel over the batch: 4Mi points are split into 8 shards of 512Ki,
one per NeuronCore; the fused table is replicated.
"""

import numpy as np

import concourse.bacc as bacc
import concourse.mybir as mybir
import concourse.tile as tile
from concourse import bass
from concourse.bass import IndirectOffsetOnAxis
from concourse.bass_utils import run_bass_kernel_spmd

# Problem constants (hardcoded per the harness contract).
N_LEVELS = 8
N_FEATS = 2
BATCH = 4194304
N_CORES = 8
PTS_PER_CORE = BATCH // N_CORES  # 524288
P = 128
FINE_RES = 2048
TBL_COLS = 32  # 16 values + 16 deltas
OUT_F = 16

# Points per partition per chunk. 256 -> 32Ki points/chunk, 16 chunks/core.
NTILE = 256

# If True, use the cast-rounding-agnostic index computation (works whether the
# f32->i32 data converter truncates or rounds-to-nearest).
SAFE_CAST = True

_nc_cache = {}


def _build_nc(pts_per_core=PTS_PER_CORE, ntile=NTILE, safe_cast=SAFE_CAST):
    """Build the per-core Bass program (SPMD: same program on every core)."""
    n_chunks, rem = divmod(pts_per_core, P * ntile)
    assert rem == 0

    nc = bacc.Bacc("TRN2", target_bir_lowering=False, debug=False)
    xs = nc.dram_tensor("xs", [pts_per_core, 1], mybir.dt.float32, kind="ExternalInput")
    gt = nc.dram_tensor("gt", [FINE_RES, TBL_COLS], mybir.dt.float32, kind="ExternalInput")
    out = nc.dram_tensor("out", [pts_per_core, OUT_F], mybir.dt.float32, kind="ExternalOutput")

    f32, i32 = mybir.dt.float32, mybir.dt.int32

    # chunk-major views: [n_chunks, 128, ntile*(1|16)]
    x_v = xs.ap().rearrange("(c p n) one -> c p (n one)", c=n_chunks, p=P, n=ntile)
    o_v = out.ap().rearrange("(c p n) f -> c p (n f)", c=n_chunks, p=P, n=ntile)

    with tile.TileContext(nc) as tc:
        with (
            tc.tile_pool(name="small", bufs=3) as sp,
            tc.tile_pool(name="gather", bufs=2) as gp,
            tc.tile_pool(name="outp", bufs=2) as op_,
        ):
            for c in range(n_chunks):
                x_t = sp.tile([P, ntile], f32, tag="x")
                nc.sync.dma_start(out=x_t[:], in_=x_v[c])

                y_t = sp.tile([P, ntile], f32, tag="y")
                nc.vector.tensor_scalar(y_t[:], x_t[:], float(FINE_RES), None, mybir.AluOpType.mult)

                m0i_t = sp.tile([P, ntile], i32, tag="m0i")
                nc.vector.tensor_copy(out=m0i_t[:], in_=y_t[:])
                m0f_t = sp.tile([P, ntile], f32, tag="m0f")
                nc.vector.tensor_copy(out=m0f_t[:], in_=m0i_t[:])
                t0_t = sp.tile([P, ntile], f32, tag="t0")
                nc.vector.tensor_sub(t0_t[:], y_t[:], m0f_t[:])

                if safe_cast:
                    # correction for whichever rounding the converter used:
                    # neg = (t0 < 0); t = t0 + neg; m = m0 - neg
                    negf_t = sp.tile([P, ntile], f32, tag="negf")
                    nc.vector.tensor_scalar(negf_t[:], t0_t[:], 0.0, None, mybir.AluOpType.is_lt)
                    t_t = sp.tile([P, ntile], f32, tag="t")
                    nc.vector.tensor_add(t_t[:], t0_t[:], negf_t[:])
                    negi_t = sp.tile([P, ntile], i32, tag="negi")
                    nc.vector.tensor_copy(out=negi_t[:], in_=negf_t[:])
                    m_t = sp.tile([P, ntile], i32, tag="m")
                    nc.vector.tensor_tensor(m_t[:], m0i_t[:], negi_t[:], mybir.AluOpType.subtract)
                else:
                    t_t = t0_t
                    m_t = m0i_t

                # gather G[m] rows: [128, ntile*32] f32, 128B per point.
                # HW DGE honors ONE dynamic offset per partition per
                # instruction (scatter_add-proven contract), so issue one
                # [128,1]-offset gather per point-column.
                g_t = gp.tile([P, ntile * TBL_COLS], f32, tag="g")
                for i in range(ntile):
                    nc.gpsimd.indirect_dma_start(
                        out=g_t[:, i * TBL_COLS : (i + 1) * TBL_COLS],
                        out_offset=None,
                        in_=gt.ap(),
                        in_offset=IndirectOffsetOnAxis(ap=m_t[:, i : i + 1], axis=0),
                    )

                g3 = g_t[:].rearrange("p (n f) -> p n f", n=ntile, f=TBL_COLS)
                g0 = g3[:, :, 0:OUT_F]
                g1 = g3[:, :, OUT_F:TBL_COLS]
                t_b = (
                    t_t[:]
                    .rearrange("p (n one) -> p n one", one=1)
                    .to_broadcast([P, ntile, OUT_F])
                )

                o_t = op_.tile([P, ntile * OUT_F], f32, tag="o")
                o3 = o_t[:].rearrange("p (n f) -> p n f", n=ntile, f=OUT_F)
                tmp_t = op_.tile([P, ntile * OUT_F], f32, tag="tmp")
                tmp3 = tmp_t[:].rearrange("p (n f) -> p n f", n=ntile, f=OUT_F)

                nc.vector.tensor_tensor(tmp3, g1, t_b, mybir.AluOpType.mult)
                nc.vector.tensor_tensor(o3, tmp3, g0, mybir.AluOpType.add)

                nc.sync.dma_start(out=o_v[c], in_=o_t[:])

    nc.compile()
    return nc


def _fuse_tables(tables: np.ndarray) -> np.ndarray:
    """Host-side fuse of the 8 per-level tables into one [2048, 32] f32 table."""
    K = FINE_RES + 1
    F = np.zeros((K, OUT_F), dtype=np.float32)
    k = np.arange(K)
    for i in range(N_LEVELS):
        res = 16 << i
        step = FINE_RES // res
        j = k // step
        u = ((k % step).astype(np.float32)) / np.float32(step)
        tj = tables[i][j]
        tj1 = tables[i][j + 1]
        F[:, 2 * i : 2 * i + 2] = tj * (1.0 - u)[:, None] + tj1 * u[:, None]
    D = (F[1:] - F[:-1]).astype(np.float32)
    return np.ascontiguousarray(np.concatenate([F[:-1], D], axis=1).astype(np.float32))


def kernel(x: np.ndarray, tables: np.ndarray, _profile: bool = False, **_run_kwargs):
    x = np.ascontiguousarray(np.asarray(x, dtype=np.float32))
    tables = np.asarray(tables, dtype=np.float32)
    assert x.shape == (BATCH, 1), x.shape

    G = _fuse_tables(tables)

    if "nc" not in _nc_cache:
        _nc_cache["nc"] = _build_nc()
    nc = _nc_cache["nc"]

    shards = x.reshape(N_CORES, PTS_PER_CORE, 1)
    in_maps = [{"xs": np.ascontiguousarray(shards[i]), "gt": G} for i in range(N_CORES)]

    res = run_bass_kernel_spmd(
        nc, in_maps, core_ids=list(range(N_CORES)), trace=_profile, **_run_kwargs
    )
    out = np.concatenate([r["out"] for r in res.results], axis=0)
    if _profile:
        kernel.last_results = res
    return out
